# revision 29
# baseline (speedup 1.0000x reference)
"""Trainium2 Bass kernel for nn_MultiHeadAttention_6219112644790.

MultiHeadAttention with structural bias lookup:
  qh/kh/vh = x @ W.T ; scores = qh*scale @ kh.T + bias_table[attn_bias]
  (255 -> -inf, global row/col -> vbias) ; softmax ; ctx @ Wo.T.

Sharding: data-parallel over batch B=8 across 8 NeuronCores (1 batch/core).

Per-core design (S=1024, H=8, D=64, HID=512), bf16 matmul datapath:
  - The additive structural bias is applied MULTIPLICATIVELY after exp:
    p = exp(s + b) = exp(s) * exp(b). The host expands ebias =
    exp(table[code]) (mask 255 -> 0.0, boundary 256 -> exp(vbias)) into
    per-(t, jc) bf16 planes; on device a single DVE tensor_mul (bf16
    2x mode) applies them. This removes the identity bias-matmul of
    the previous design (~82k PE cycles).
  - scores computed transposed, sT[j, i] per head pair, from compact
    bf16 qhT (head-padded) / khT layouts.
  - ctx~T[d, i] = sum_j vh[j, d] * ptm[j, i]; an appended ones-column
    of vhA yields Z (softmax denominator) as PSUM row 64.
  - per t-column: ctx+Z evicted via 4 wide DVE copies into one staging
    tile, then 2 remap DMAs + 2 Z DMAs (SBUF->SBUF). 1/Z reciprocal on
    DVE (approx), bf16 1/Z broadcast via a tiny [2, 128] pair-mask
    matmul into a PSUM tile; the reciprocal, division and output
    projection of column t-1 are interleaved into column t's attention
    loop at fixed jc offsets so no engine stream blocks.
"""

import numpy as np

import concourse.bacc as bacc
import concourse.mybir as mybir
import concourse.tile as tile
from concourse.bass_utils import run_bass_kernel_spmd

F32 = mybir.dt.float32
F32R = mybir.dt.float32r
BF16 = mybir.dt.bfloat16

B, S, HID, H, D = 8, 1024, 512, 8, 64
N = S - 1  # interior sequence positions; index S-1 is the global node
NE = 257   # table entries: 255 real codes + mask(255) + boundary(256)
SCALE = float(D) ** -0.5

_CACHE = {}


# ----------------------------------------------------------------- device ---

def build_nc(num_devices=8, debug=False):
    nc = bacc.Bacc("TRN2", target_bir_lowering=False, debug=False,
                   num_devices=num_devices)
    q_d = nc.dram_tensor("q", [S, HID], BF16, kind="ExternalInput")
    k_d = nc.dram_tensor("k", [S, HID], BF16, kind="ExternalInput")
    v_d = nc.dram_tensor("v", [S, HID], BF16, kind="ExternalInput")
    wexp_d = nc.dram_tensor("wexp", [8, 128, 8192], BF16, kind="ExternalInput")
    wq_d = nc.dram_tensor("wq", [HID, HID], BF16, kind="ExternalInput")
    wk_d = nc.dram_tensor("wk", [HID, HID], BF16, kind="ExternalInput")
    wv_d = nc.dram_tensor("wv", [HID, HID], BF16, kind="ExternalInput")
    wo_d = nc.dram_tensor("wo", [HID, HID], F32R, kind="ExternalInput")
    id16_d = nc.dram_tensor("ident16", [128, 128], BF16, kind="ExternalInput")
    pmask_d = nc.dram_tensor("pmask", [2, 128], BF16, kind="ExternalInput")
    out_d = nc.dram_tensor("out", [S, HID], F32, kind="ExternalOutput")
    dbg = {}

    with tile.TileContext(nc) as tc:
        _emit(nc, tc, q_d, k_d, v_d, wexp_d, wq_d, wk_d, wv_d, wo_d,
              id16_d, pmask_d, out_d, dbg)
    nc.compile()
    return nc


def _emit(nc, tc, q_d, k_d, v_d, wexp_d, wq_d, wk_d, wv_d, wo_d, id16_d,
          pmask_d, out_d, dbg):
    from contextlib import ExitStack
    ctx_mgr = ExitStack()
    with ctx_mgr:
        P = lambda **kw: ctx_mgr.enter_context(tc.tile_pool(**kw))
        const = P(name="const", bufs=1)
        persist = P(name="persist", bufs=1)
        wxp = P(name="wexp", bufs=6)
        ptp = P(name="ptraw", bufs=3)
        pmp = P(name="ptm", bufs=3)
        stp = P(name="stgw", bufs=2)
        outp = P(name="outp", bufs=2)
        xbp = P(name="xpersist", bufs=1)

        # ---- constants
        id16_t = const.tile([128, 128], BF16)
        nc.sync.dma_start(id16_t[:], id16_d[:])
        pmask_t = const.tile([2, 128], BF16)
        nc.sync.dma_start(pmask_t[:], pmask_d[:])
        wo_t = const.tile([128, 4, 512], F32R, tag="w_wo")

        qhT = persist.tile([128, 8, 1024], BF16, tag="qhT")
        khT = persist.tile([128, 4, 1024], BF16, tag="khT")
        vhA = persist.tile([128, 8, 520], BF16, tag="vhA")
        ctx_sb = persist.tile([128, 4, 1024], F32R, tag="ctx")
        zc2 = persist.tile([2, 1024], F32, tag="zc2")
        zr2 = persist.tile([2, 1024], F32, tag="zr2")
        zr16 = persist.tile([2, 1024], BF16, tag="zr16")
        # zero the head-padded halves of qhT; the ones column of vhA.
        # On GpSimd: keeps the big memset out of the DVE in-order stream.
        nc.gpsimd.memset(qhT[:], 0.0)
        nc.gpsimd.memset(
            vhA[:].rearrange("p sc (h dd) -> p sc h dd", dd=65)[:, :, :, 64:65],
            1.0)

        # ---- phase A: all transposes first (psT), then projections from
        # the psS ring; q's second-half projection and the v projection are
        # deferred into column t0 of phase B so exp starts ~15 us earlier.
        xts = {}
        wts = {}
        with (tc.tile_pool(name="psT", bufs=4, space="PSUM") as psT,
              tc.tile_pool(name="psS", bufs=2, space="PSUM") as psS_a,
              tc.tile_pool(name="qn", bufs=4) as qn_pool,
              tc.tile_pool(name="kdie", bufs=1) as kdie_pool):
            psS = psS_a

            def emit_transposes(nm, src):
                src_r = src[:].rearrange("(sc p) e -> p sc e", p=128)
                xT = xts[nm]
                for sg in range(2):
                    qn = qn_pool.tile([128, 2, 2, 512], BF16, tag="qn",
                                      name=f"qn_{nm}{sg}")
                    nc.sync.dma_start(qn[:, 0], src_r[:, 4 * sg:4 * sg + 2, :])
                    nc.sync.dma_start(qn[:, 1], src_r[:, 4 * sg + 2:4 * sg + 4, :])
                    pts = [psT.tile([128, 512], BF16, tag="pst",
                                    name=f"pts_{nm}{sg}_{_i}") for _i in range(4)]
                    for s4 in range(4):
                        for cb in range(4):
                            nc.tensor.transpose(
                                pts[cb][:, 128 * s4:128 * s4 + 128],
                                qn[:, s4 // 2, s4 % 2, 128 * cb:128 * cb + 128],
                                id16_t[:])
                    for cb in range(4):
                        nc.vector.tensor_copy(xT[:, cb, 512 * sg:512 * sg + 512],
                                              pts[cb][:])

            def emit_kqproj(nm, nhs):
                w_t, xT = wts[nm], xts[nm]
                for ech in range(4):
                    for nh in nhs:
                        pp = psS.tile([128, 1024], F32, tag="sc",
                                      name=f"pp_{nm}{ech}{nh}")
                        for kk in range(4):
                            nc.tensor.matmul(
                                pp[:, 0:512],
                                w_t[:, kk, 128 * ech:128 * ech + 128],
                                xT[:, kk, 512 * nh:512 * nh + 512],
                                start=(kk == 0), stop=(kk == 3))
                        if nm == "k":
                            nc.scalar.copy(
                                khT[:, ech, 512 * nh:512 * nh + 512],
                                pp[:, 0:512])
                        else:
                            # head-padded: head h at chunk h, partition
                            # half 64*(h%2); the other half stays zero
                            nc.scalar.copy(
                                qhT[0:64, 2 * ech, 512 * nh:512 * nh + 512],
                                pp[0:64, 0:512])
                            nc.scalar.copy(
                                qhT[64:128, 2 * ech + 1, 512 * nh:512 * nh + 512],
                                pp[64:128, 0:512])

            for nm, src, wsrc in (("k", k_d, wk_d), ("q", q_d, wq_d),
                                  ("v", v_d, wv_d)):
                pool = kdie_pool if nm == "k" else xbp
                wts[nm] = pool.tile([128, 4, 512], BF16, tag=f"w_{nm}",
                                    name=f"w_{nm}")
                nc.sync.dma_start(
                    wts[nm][:], wsrc[:].rearrange("(kk p) e -> p kk e", p=128))
                xts[nm] = pool.tile([128, 4, 1024], BF16, tag=f"xT_{nm}",
                                    name=f"xT_{nm}")
            emit_transposes("k", k_d)
            emit_kqproj("k", (0, 1))
            emit_transposes("q", q_d)
            emit_kqproj("q", (0,))
            emit_transposes("v", v_d)
        xT_q, w_q = xts["q"], wts["q"]
        xT_v, w_v = xts["v"], wts["v"]
        # wo load late (consumed only by the output projection)
        nc.sync.dma_start(wo_t[:], wo_d[:].rearrange("(kk p) e -> p kk e", p=128))

        # ---- phase B: attention with interleaved normalize + out-proj --------
        out_r = out_d[:].rearrange("(sc p) e -> p sc e", p=128)

        with (tc.tile_pool(name="psS2", bufs=2, space="PSUM") as psS,
              tc.tile_pool(name="psC", bufs=4, space="PSUM") as psC):

            def emit_vproj(sc):
                pp = psS.tile([128, 1024], F32, tag="sc", name=f"vp{sc}")
                for kk in range(4):
                    nc.tensor.matmul(
                        pp[:, 0:512],
                        xT_v[:, kk, 128 * sc:128 * sc + 128],
                        w_v[:, kk, :],
                        start=(kk == 0), stop=(kk == 3))
                nc.vector.tensor_copy(
                    vhA[:, sc, :].rearrange("p (h dd) -> p h dd", dd=65)[:, :, 0:64],
                    pp[:, 0:512].rearrange("p (h dd) -> p h dd", dd=64))

            def emit_qproj1(ech):
                pp = psS.tile([128, 1024], F32, tag="sc", name=f"qp{ech}")
                for kk in range(4):
                    nc.tensor.matmul(
                        pp[:, 0:512],
                        w_q[:, kk, 128 * ech:128 * ech + 128],
                        xT_q[:, kk, 512:1024],
                        start=(kk == 0), stop=(kk == 3))
                nc.vector.tensor_copy(
                    qhT[0:64, 2 * ech, 512:1024], pp[0:64, 0:512])
                nc.vector.tensor_copy(
                    qhT[64:128, 2 * ech + 1, 512:1024], pp[64:128, 0:512])

            def emit_recip(tp):
                nc.vector.reciprocal_approx_fast(zr2[:], zc2[:])
                nc.vector.tensor_copy(zr16[:], zr2[:])

            def emit_rb(tp):
                # 1/Z broadcast: rb[64*s:64*s+64, 256*m:...] = zr16[s, blk]
                # via a [2, 128] pair-mask bf16 matmul (tiny)
                rb = psS.tile([128, 1024], F32, tag="sc", name=f"rb{tp}")
                for m in range(4):
                    nc.tensor.matmul(
                        rb[:, 256 * m:256 * m + 256],
                        pmask_t[:],
                        zr16[0:2, 256 * m:256 * m + 256],
                        start=True, stop=True)
                return rb

            def emit_div(tp, rb):
                for m in range(4):
                    nc.vector.tensor_mul(
                        ctx_sb[:, m, 256 * tp:256 * tp + 256],
                        ctx_sb[:, m, 256 * tp:256 * tp + 256],
                        rb[:, 256 * m:256 * m + 256])

            def emit_outproj(tp):
                po = psS.tile([128, 1024], F32, tag="sc", name=f"po{tp}")
                for half in range(2):
                    sc = 2 * tp + half
                    for ech in range(4):
                        nc.tensor.matmul(
                            po[:, 512 * half:512 * half + 512],
                            ctx_sb[:, ech, 128 * sc:128 * sc + 128],
                            wo_t[:, ech, :],
                            start=(ech == 0), stop=(ech == 3))
                ot = outp.tile([128, 2, 512], F32, tag="o", name=f"ot{tp}")
                nc.vector.tensor_copy(ot[:], po[:])
                nc.sync.dma_start(out_r[:, 2 * tp:2 * tp + 2, :], ot[:])

            for t in range(4):
                ctx_ps = [psC.tile([128, 512], F32, tag="ctxps",
                                   name=f"ctxps{t}_{_i}") for _i in range(4)]
                rb_prev = None
                if t == 0:
                    emit_vproj(0)
                    emit_vproj(1)
                for jc in range(8):
                    wt = wxp.tile([128, 2048], BF16, tag="wexp",
                                  name=f"wt{t}_{jc}")
                    nc.gpsimd.dma_start(
                        wt[:], wexp_d[jc][:, 2048 * t:2048 * t + 2048])
                    for g in range(2):
                        ps = psS.tile([128, 1024], F32, tag="sc",
                                      name=f"ps{t}_{jc}_{g}")
                        for gp in range(2):
                            h0 = 4 * g + 2 * gp  # head pair (h0, h0+1)
                            nc.tensor.matmul(
                                ps[:, 512 * gp:512 * gp + 512],
                                khT[:, h0 // 2, 128 * jc:128 * jc + 128],
                                qhT[:, h0:h0 + 2, 256 * t:256 * t + 256],
                                start=True, stop=True)
                        pt_raw = ptp.tile([128, 1024], BF16, tag="ptr",
                                          name=f"ptr{t}_{jc}_{g}")
                        nc.scalar.activation(pt_raw[:], ps[:],
                                             mybir.ActivationFunctionType.Exp)
                        ptm = pmp.tile([128, 1024], BF16, tag="ptm",
                                       name=f"ptm{t}_{jc}_{g}")
                        nc.vector.tensor_mul(ptm[:], pt_raw[:],
                                             wt[:, 1024 * g:1024 * g + 1024])
                        for hl in range(4):
                            h = 4 * g + hl
                            bank, side = h // 2, h % 2
                            nc.tensor.matmul(
                                ctx_ps[bank][0:65, 256 * side:256 * side + 256],
                                vhA[:, jc, 65 * h:65 * h + 65],
                                ptm[:, 256 * hl:256 * hl + 256],
                                start=(jc == 0 and side == 0),
                                stop=(jc == 7 and side == 1))
                    if t == 0:
                        if jc < 6:
                            emit_vproj(jc + 2)
                        if 3 <= jc <= 6:
                            emit_qproj1(jc - 3)
                    if t > 0:
                        # deferred normalize chain of column t-1, spread at
                        # fixed jc offsets: recip+cvt early (DVE stream must
                        # not block on the Z DMA), rb+div together (div must
                        # precede the next mult in the DVE stream or the psS
                        # ring WAR deadlocks), out-projection last
                        if jc == 1:
                            emit_recip(t - 1)
                        elif jc == 4:
                            rb_prev = emit_rb(t - 1)
                            emit_div(t - 1, rb_prev)
                        elif jc == 6:
                            emit_outproj(t - 1)
                # evict ctx + Z for this t: 4 wide DVE copies into one
                # staging tile, then 2 remap DMAs + 2 Z DMAs
                stgW = stp.tile([65, 2048], F32, tag="stgw", name=f"stgW{t}")
                for bank in range(4):
                    nc.vector.tensor_copy(stgW[:, 512 * bank:512 * bank + 512],
                                          ctx_ps[bank][0:65, :])
                stg_r = stgW[0:64, :].rearrange("p (b s c) -> p b s c", b=4, s=2)
                for side in range(2):
                    nc.sync.dma_start(
                        ctx_sb[64 * side:64 * side + 64, :,
                               256 * t:256 * t + 256].bitcast(F32),
                        stg_r[:, :, side, :])
                zrow = stgW[64:65, :].rearrange("p (b s c) -> p s b c", b=4, s=2)
                for side in range(2):
                    for bh in range(2):
                        nc.sync.dma_start(
                            zc2[side:side + 1, 512 * bh:512 * bh + 512],
                            zrow[:, side, 2 * bh:2 * bh + 2])
            # tail: last column's normalize + projection, pipelined per
            # head-pair m so recip/rb/div overlap across banks
            rb3 = psS.tile([128, 1024], F32, tag="sc", name="rb3")
            for m in range(4):
                blk = 256 * m
                nc.vector.reciprocal_approx_fast(zr2[:, blk:blk + 256],
                                                 zc2[:, blk:blk + 256])
                nc.vector.tensor_copy(zr16[:, blk:blk + 256],
                                      zr2[:, blk:blk + 256])
                nc.tensor.matmul(
                    rb3[:, 256 * m:256 * m + 256],
                    pmask_t[:],
                    zr16[0:2, blk:blk + 256],
                    start=True, stop=True)
                nc.vector.tensor_mul(
                    ctx_sb[:, m, 768:1024],
                    ctx_sb[:, m, 768:1024],
                    rb3[:, 256 * m:256 * m + 256])
            emit_outproj(3)


# ------------------------------------------------------------------- host ---

def _host_prep(inputs):
    import ml_dtypes
    bf16 = ml_dtypes.bfloat16
    q = np.asarray(inputs["q"], dtype=np.float32).astype(bf16)
    k = np.asarray(inputs["k"], dtype=np.float32).astype(bf16)
    v = np.asarray(inputs["v"], dtype=np.float32).astype(bf16)
    ab = np.asarray(inputs["attn_bias"])[:, :, :, 0]  # [B, N, N] int32
    for bn in ("bq", "bk", "bv", "bo"):
        assert not np.any(np.asarray(inputs[bn])), f"nonzero bias {bn} unsupported"

    wq = np.ascontiguousarray(
        (SCALE * np.asarray(inputs["Wq"], np.float32)).T).astype(bf16)
    wk = np.ascontiguousarray(np.asarray(inputs["Wk"], np.float32).T).astype(bf16)
    wv = np.ascontiguousarray(np.asarray(inputs["Wv"], np.float32).T).astype(bf16)
    wo = np.ascontiguousarray(np.asarray(inputs["Wo"], np.float32).T)

    Tp = np.zeros((NE, H), np.float32)
    Tp[:256] = np.asarray(inputs["bias_table"], np.float32)
    Tp[256] = np.asarray(inputs["vbias"], np.float32)[0]
    E = np.exp(Tp)
    E[255] = 0.0  # masked -> exp(-inf)
    E16 = E.astype(bf16)

    ident16 = np.eye(128, dtype=bf16)
    pmask = np.zeros((2, 128), bf16)
    pmask[0, 0:64] = 1.0
    pmask[1, 64:128] = 1.0

    in_maps = []
    for b in range(B):
        cpad = np.full((1024, 1024), 256, np.int64)
        cpad[:N, :N] = ab[b].T  # cpad[j, i] = ab[b, i, j]
        W = E16[cpad]  # [1024 j, 1024 i, 8 h] multiplicative bias planes
        wexp = np.ascontiguousarray(
            W.reshape(8, 128, 4, 256, 2, 4).transpose(0, 1, 2, 4, 5, 3)
            .reshape(8, 128, 8192))
        in_maps.append({
            "q": q[b], "k": k[b], "v": v[b], "wexp": wexp,
            "wq": wq, "wk": wk, "wv": wv, "wo": wo,
            "ident16": ident16, "pmask": pmask,
        })
    return in_maps


def _run(inputs, trace=False, **kw):
    in_maps = _host_prep(inputs)
    if "nc8" not in _CACHE:
        _CACHE["nc8"] = build_nc(num_devices=8, debug=False)
    res = run_bass_kernel_spmd(_CACHE["nc8"], in_maps, core_ids=list(range(8)),
                               trace=trace, **kw)
    return np.stack([r["out"] for r in res.results], axis=0), res


def kernel(**inputs) -> np.ndarray:
    out, _ = _run(inputs)
    return out


# revision 30
# speedup vs baseline: 1.2087x; 1.2087x over previous
"""Trainium2 Bass kernel for nn_MultiHeadAttention_6219112644790.

MultiHeadAttention with structural bias lookup:
  qh/kh/vh = x @ W.T ; scores = qh*scale @ kh.T + bias_table[attn_bias]
  (255 -> -inf, global row/col -> vbias) ; softmax ; ctx @ Wo.T.

Sharding: data-parallel over batch B=8 across 8 NeuronCores (1 batch/core).

Per-core design (S=1024, H=8, D=64, HID=512), bf16 matmul datapath:
  - The additive structural bias is applied MULTIPLICATIVELY after exp:
    p = exp(s + b) = exp(s) * exp(b). The host expands ebias =
    exp(table[code]) (mask 255 -> 0.0, boundary 256 -> exp(vbias)) into
    per-(t, jc) bf16 planes; on device a single DVE tensor_mul (bf16
    2x mode) applies them. This removes the identity bias-matmul of
    the previous design (~82k PE cycles).
  - scores computed transposed, sT[j, i] per head pair, from compact
    bf16 qhT (head-padded) / khT layouts.
  - ctx~T[d, i] = sum_j vh[j, d] * ptm[j, i]; an appended ones-column
    of vhA yields Z (softmax denominator) as PSUM row 64.
  - per t-column: ctx+Z evicted via 4 wide DVE copies into one staging
    tile, then 2 remap DMAs + 2 Z DMAs (SBUF->SBUF). 1/Z reciprocal on
    DVE (approx), bf16 1/Z broadcast via a tiny [2, 128] pair-mask
    matmul into a PSUM tile; the reciprocal, division and output
    projection of column t-1 are interleaved into column t's attention
    loop at fixed jc offsets so no engine stream blocks.
"""

import numpy as np

import concourse.bacc as bacc
import concourse.mybir as mybir
import concourse.tile as tile
from concourse.bass_utils import run_bass_kernel_spmd

F32 = mybir.dt.float32
F32R = mybir.dt.float32r
BF16 = mybir.dt.bfloat16

B, S, HID, H, D = 8, 1024, 512, 8, 64
N = S - 1  # interior sequence positions; index S-1 is the global node
NE = 257   # table entries: 255 real codes + mask(255) + boundary(256)
SCALE = float(D) ** -0.5

_CACHE = {}


# ----------------------------------------------------------------- device ---

def build_nc(num_devices=8, debug=False):
    nc = bacc.Bacc("TRN2", target_bir_lowering=False, debug=False,
                   num_devices=num_devices)
    q_d = nc.dram_tensor("q", [S, HID], BF16, kind="ExternalInput")
    k_d = nc.dram_tensor("k", [S, HID], BF16, kind="ExternalInput")
    v_d = nc.dram_tensor("v", [S, HID], BF16, kind="ExternalInput")
    wexp_d = nc.dram_tensor("wexp", [8, 128, 8192], BF16, kind="ExternalInput")
    wq_d = nc.dram_tensor("wq", [HID, HID], BF16, kind="ExternalInput")
    wk_d = nc.dram_tensor("wk", [HID, HID], BF16, kind="ExternalInput")
    wv_d = nc.dram_tensor("wv", [HID, HID], BF16, kind="ExternalInput")
    wo_d = nc.dram_tensor("wo", [HID, HID], F32R, kind="ExternalInput")
    id16_d = nc.dram_tensor("ident16", [128, 128], BF16, kind="ExternalInput")
    pmask_d = nc.dram_tensor("pmask", [2, 128], BF16, kind="ExternalInput")
    out_d = nc.dram_tensor("out", [S, HID], F32, kind="ExternalOutput")
    dbg = {}

    with tile.TileContext(nc) as tc:
        _emit(nc, tc, q_d, k_d, v_d, wexp_d, wq_d, wk_d, wv_d, wo_d,
              id16_d, pmask_d, out_d, dbg)
    nc.compile()
    return nc


def _emit(nc, tc, q_d, k_d, v_d, wexp_d, wq_d, wk_d, wv_d, wo_d, id16_d,
          pmask_d, out_d, dbg):
    from contextlib import ExitStack
    ctx_mgr = ExitStack()
    with ctx_mgr:
        P = lambda **kw: ctx_mgr.enter_context(tc.tile_pool(**kw))
        const = P(name="const", bufs=1)
        persist = P(name="persist", bufs=1)
        wxp = P(name="wexp", bufs=6)
        ptp = P(name="ptraw", bufs=3)
        pmp = P(name="ptm", bufs=3)
        stp = P(name="stgw", bufs=2)
        outp = P(name="outp", bufs=2)

        # ---- constants
        id16_t = const.tile([128, 128], BF16)
        nc.sync.dma_start(id16_t[:], id16_d[:])
        pmask_t = const.tile([2, 128], BF16)
        nc.sync.dma_start(pmask_t[:], pmask_d[:])
        wo_t = const.tile([128, 4, 512], F32R, tag="w_wo")

        qhT = persist.tile([128, 8, 1024], BF16, tag="qhT")
        khT = persist.tile([128, 4, 1024], BF16, tag="khT")
        vhA = persist.tile([128, 8, 520], BF16, tag="vhA")
        ctx_sb = persist.tile([128, 4, 1024], F32R, tag="ctx")
        zc2 = persist.tile([2, 4096], F32, tag="zc2")
        zr2 = persist.tile([2, 4096], F32, tag="zr2")
        zr16 = persist.tile([2, 4096], BF16, tag="zr16")
        # zero the head-padded halves of qhT; the ones column of vhA.
        # On GpSimd: keeps the big memset out of the DVE in-order stream.
        nc.gpsimd.memset(qhT[:], 0.0)
        nc.gpsimd.memset(
            vhA[:].rearrange("p sc (h dd) -> p sc h dd", dd=65)[:, :, :, 64:65],
            1.0)

        # ---- phase A: transposes + projections -------------------------------
        with (tc.tile_pool(name="psT", bufs=4, space="PSUM") as psT,
              tc.tile_pool(name="psA", bufs=4, space="PSUM") as psA,
              tc.tile_pool(name="qn", bufs=4) as qn_pool,
              tc.tile_pool(name="xT", bufs=1) as xT_pool,
              tc.tile_pool(name="wqkv", bufs=2) as wqkv_pool):
            for nm, src, wsrc in (("k", k_d, wk_d), ("q", q_d, wq_d),
                                  ("v", v_d, wv_d)):
                w_t = wqkv_pool.tile([128, 4, 512], BF16, tag="w_in",
                                     name=f"w_{nm}")
                nc.sync.dma_start(w_t[:], wsrc[:].rearrange("(kk p) e -> p kk e", p=128))
                xT = xT_pool.tile([128, 4, 1024], BF16, tag="xT",
                                  name=f"xT_{nm}")
                src_r = src[:].rearrange("(sc p) e -> p sc e", p=128)
                for sg in range(2):
                    # double-chunk load: seq rows [512*sg, 512*sg+512)
                    qn = qn_pool.tile([128, 2, 2, 512], BF16, tag="qn",
                                      name=f"qn_{nm}{sg}")
                    nc.sync.dma_start(qn[:, 0], src_r[:, 4 * sg:4 * sg + 2, :])
                    nc.sync.dma_start(qn[:, 1], src_r[:, 4 * sg + 2:4 * sg + 4, :])
                    pts = [psT.tile([128, 512], BF16, tag="pst",
                                    name=f"pts_{nm}{sg}_{_i}") for _i in range(4)]
                    for s4 in range(4):
                        for cb in range(4):
                            nc.tensor.transpose(
                                pts[cb][:, 128 * s4:128 * s4 + 128],
                                qn[:, s4 // 2, s4 % 2, 128 * cb:128 * cb + 128],
                                id16_t[:])
                    for cb in range(4):
                        nc.vector.tensor_copy(xT[:, cb, 512 * sg:512 * sg + 512],
                                              pts[cb][:])
                if nm in ("q", "k"):
                    for ech in range(4):
                        for nh in range(2):
                            pp = psA.tile([128, 512], F32, tag="psa",
                                          name=f"pp_{nm}{ech}{nh}")
                            for kk in range(4):
                                nc.tensor.matmul(
                                    pp[:],
                                    w_t[:, kk, 128 * ech:128 * ech + 128],
                                    xT[:, kk, 512 * nh:512 * nh + 512],
                                    start=(kk == 0), stop=(kk == 3))
                            if nm == "k":
                                nc.scalar.copy(
                                    khT[:, ech, 512 * nh:512 * nh + 512], pp[:])
                            else:
                                # head-padded: head h at chunk h, partition
                                # half 64*(h%2); the other half stays zero
                                nc.scalar.copy(
                                    qhT[0:64, 2 * ech, 512 * nh:512 * nh + 512],
                                    pp[0:64, :])
                                nc.scalar.copy(
                                    qhT[64:128, 2 * ech + 1, 512 * nh:512 * nh + 512],
                                    pp[64:128, :])
                else:
                    for sc in range(8):
                        pp = psA.tile([128, 512], F32, tag="psa",
                                      name=f"pp_v{sc}")
                        for kk in range(4):
                            nc.tensor.matmul(
                                pp[:],
                                xT[:, kk, 128 * sc:128 * sc + 128],
                                w_t[:, kk, :],
                                start=(kk == 0), stop=(kk == 3))
                        nc.scalar.copy(
                            vhA[:, sc, :].rearrange("p (h dd) -> p h dd", dd=65)[:, :, 0:64],
                            pp[:].rearrange("p (h dd) -> p h dd", dd=64))
        # wo load late (consumed only by the output projection)
        nc.sync.dma_start(wo_t[:], wo_d[:].rearrange("(kk p) e -> p kk e", p=128))

        # ---- phase B: attention with interleaved normalize + out-proj --------
        out_r = out_d[:].rearrange("(sc p) e -> p sc e", p=128)

        with (tc.tile_pool(name="psS", bufs=2, space="PSUM") as psS,
              tc.tile_pool(name="psC", bufs=4, space="PSUM") as psC):

            def emit_recip(tp):
                nc.vector.reciprocal_approx_fast(
                    zr2[:, 1024 * tp:1024 * tp + 1024],
                    zc2[:, 1024 * tp:1024 * tp + 1024])
                nc.vector.tensor_copy(zr16[:, 1024 * tp:1024 * tp + 1024],
                                      zr2[:, 1024 * tp:1024 * tp + 1024])

            def emit_rb(tp):
                # 1/Z broadcast: rb[64*s:64*s+64, 256*m:...] = zr16[s, blk]
                # via a [2, 128] pair-mask bf16 matmul (tiny)
                rb = psS.tile([128, 1024], F32, tag="sc", name=f"rb{tp}")
                for m in range(4):
                    blk = (4 * tp + m) * 256
                    nc.tensor.matmul(
                        rb[:, 256 * m:256 * m + 256],
                        pmask_t[:],
                        zr16[0:2, blk:blk + 256],
                        start=True, stop=True)
                return rb

            def emit_div(tp, rb):
                for m in range(4):
                    nc.vector.tensor_mul(
                        ctx_sb[:, m, 256 * tp:256 * tp + 256],
                        ctx_sb[:, m, 256 * tp:256 * tp + 256],
                        rb[:, 256 * m:256 * m + 256])

            def emit_outproj(tp):
                po = psS.tile([128, 1024], F32, tag="sc", name=f"po{tp}")
                for half in range(2):
                    sc = 2 * tp + half
                    for ech in range(4):
                        nc.tensor.matmul(
                            po[:, 512 * half:512 * half + 512],
                            ctx_sb[:, ech, 128 * sc:128 * sc + 128],
                            wo_t[:, ech, :],
                            start=(ech == 0), stop=(ech == 3))
                ot = outp.tile([128, 2, 512], F32, tag="o", name=f"ot{tp}")
                nc.vector.tensor_copy(ot[:], po[:])
                nc.sync.dma_start(out_r[:, 2 * tp:2 * tp + 2, :], ot[:])

            for t in range(4):
                ctx_ps = [psC.tile([128, 512], F32, tag="ctxps",
                                   name=f"ctxps{t}_{_i}") for _i in range(4)]
                rb_prev = None
                for jc in range(8):
                    wt = wxp.tile([128, 2048], BF16, tag="wexp",
                                  name=f"wt{t}_{jc}")
                    nc.gpsimd.dma_start(
                        wt[:], wexp_d[jc][:, 2048 * t:2048 * t + 2048])
                    for g in range(2):
                        ps = psS.tile([128, 1024], F32, tag="sc",
                                      name=f"ps{t}_{jc}_{g}")
                        for gp in range(2):
                            h0 = 4 * g + 2 * gp  # head pair (h0, h0+1)
                            nc.tensor.matmul(
                                ps[:, 512 * gp:512 * gp + 512],
                                khT[:, h0 // 2, 128 * jc:128 * jc + 128],
                                qhT[:, h0:h0 + 2, 256 * t:256 * t + 256],
                                start=True, stop=True)
                        pt_raw = ptp.tile([128, 1024], BF16, tag="ptr",
                                          name=f"ptr{t}_{jc}_{g}")
                        nc.scalar.activation(pt_raw[:], ps[:],
                                             mybir.ActivationFunctionType.Exp)
                        ptm = pmp.tile([128, 1024], BF16, tag="ptm",
                                       name=f"ptm{t}_{jc}_{g}")
                        nc.vector.tensor_mul(ptm[:], pt_raw[:],
                                             wt[:, 1024 * g:1024 * g + 1024])
                        for hl in range(4):
                            h = 4 * g + hl
                            bank, side = h // 2, h % 2
                            nc.tensor.matmul(
                                ctx_ps[bank][0:65, 256 * side:256 * side + 256],
                                vhA[:, jc, 65 * h:65 * h + 65],
                                ptm[:, 256 * hl:256 * hl + 256],
                                start=(jc == 0 and side == 0),
                                stop=(jc == 7 and side == 1))
                    if t > 0:
                        # deferred normalize chain of column t-1, spread at
                        # fixed jc offsets: recip+cvt early (DVE stream must
                        # not block on the Z DMA), rb+div together (div must
                        # precede the next mult in the DVE stream or the psS
                        # ring WAR deadlocks), out-projection last
                        if jc == 1:
                            emit_recip(t - 1)
                        elif jc == 4:
                            rb_prev = emit_rb(t - 1)
                            emit_div(t - 1, rb_prev)
                        elif jc == 6:
                            emit_outproj(t - 1)
                # evict ctx + Z for this t: 4 wide DVE copies into one
                # staging tile, then 2 remap DMAs + 2 Z DMAs
                stgW = stp.tile([65, 2048], F32, tag="stgw", name=f"stgW{t}")
                for bank in range(4):
                    nc.vector.tensor_copy(stgW[:, 512 * bank:512 * bank + 512],
                                          ctx_ps[bank][0:65, :])
                stg_r = stgW[0:64, :].rearrange("p (b s c) -> p b s c", b=4, s=2)
                for side in range(2):
                    nc.sync.dma_start(
                        ctx_sb[64 * side:64 * side + 64, :,
                               256 * t:256 * t + 256].bitcast(F32),
                        stg_r[:, :, side, :])
                zrow = stgW[64:65, :].rearrange("p (b s c) -> p s b c", b=4, s=2)
                for side in range(2):
                    for bh in range(2):
                        nc.sync.dma_start(
                            zc2[side:side + 1,
                                1024 * t + 512 * bh:1024 * t + 512 * bh + 512],
                            zrow[:, side, 2 * bh:2 * bh + 2]),
            # tail: last column's normalize + projection, pipelined per
            # head-pair m so recip/rb/div overlap across banks
            rb3 = psS.tile([128, 1024], F32, tag="sc", name="rb3")
            for m in range(4):
                blk = (12 + m) * 256
                nc.vector.reciprocal_approx_fast(zr2[:, blk:blk + 256],
                                                 zc2[:, blk:blk + 256])
                nc.vector.tensor_copy(zr16[:, blk:blk + 256],
                                      zr2[:, blk:blk + 256])
                nc.tensor.matmul(
                    rb3[:, 256 * m:256 * m + 256],
                    pmask_t[:],
                    zr16[0:2, blk:blk + 256],
                    start=True, stop=True)
                nc.vector.tensor_mul(
                    ctx_sb[:, m, 768:1024],
                    ctx_sb[:, m, 768:1024],
                    rb3[:, 256 * m:256 * m + 256])
            emit_outproj(3)


# ------------------------------------------------------------------- host ---

def _host_prep(inputs):
    import ml_dtypes
    bf16 = ml_dtypes.bfloat16
    q = np.asarray(inputs["q"], dtype=np.float32).astype(bf16)
    k = np.asarray(inputs["k"], dtype=np.float32).astype(bf16)
    v = np.asarray(inputs["v"], dtype=np.float32).astype(bf16)
    ab = np.asarray(inputs["attn_bias"])[:, :, :, 0]  # [B, N, N] int32
    for bn in ("bq", "bk", "bv", "bo"):
        assert not np.any(np.asarray(inputs[bn])), f"nonzero bias {bn} unsupported"

    wq = np.ascontiguousarray(
        (SCALE * np.asarray(inputs["Wq"], np.float32)).T).astype(bf16)
    wk = np.ascontiguousarray(np.asarray(inputs["Wk"], np.float32).T).astype(bf16)
    wv = np.ascontiguousarray(np.asarray(inputs["Wv"], np.float32).T).astype(bf16)
    wo = np.ascontiguousarray(np.asarray(inputs["Wo"], np.float32).T)

    Tp = np.zeros((NE, H), np.float32)
    Tp[:256] = np.asarray(inputs["bias_table"], np.float32)
    Tp[256] = np.asarray(inputs["vbias"], np.float32)[0]
    E = np.exp(Tp)
    E[255] = 0.0  # masked -> exp(-inf)
    E16 = E.astype(bf16)

    ident16 = np.eye(128, dtype=bf16)
    pmask = np.zeros((2, 128), bf16)
    pmask[0, 0:64] = 1.0
    pmask[1, 64:128] = 1.0

    in_maps = []
    for b in range(B):
        cpad = np.full((1024, 1024), 256, np.int64)
        cpad[:N, :N] = ab[b].T  # cpad[j, i] = ab[b, i, j]
        W = E16[cpad]  # [1024 j, 1024 i, 8 h] multiplicative bias planes
        wexp = np.ascontiguousarray(
            W.reshape(8, 128, 4, 256, 2, 4).transpose(0, 1, 2, 4, 5, 3)
            .reshape(8, 128, 8192))
        in_maps.append({
            "q": q[b], "k": k[b], "v": v[b], "wexp": wexp,
            "wq": wq, "wk": wk, "wv": wv, "wo": wo,
            "ident16": ident16, "pmask": pmask,
        })
    return in_maps


def _run(inputs, trace=False, **kw):
    in_maps = _host_prep(inputs)
    if "nc8" not in _CACHE:
        _CACHE["nc8"] = build_nc(num_devices=8, debug=False)
    res = run_bass_kernel_spmd(_CACHE["nc8"], in_maps, core_ids=list(range(8)),
                               trace=trace, **kw)
    return np.stack([r["out"] for r in res.results], axis=0), res


def kernel(**inputs) -> np.ndarray:
    out, _ = _run(inputs)
    return out
